# revision 13
# baseline (speedup 1.0000x reference)
"""Trainium2 Bass kernel for a single causal-attention transformer block.

Reference computation (per batch element b):
    xn  = rms_norm(x[b]) * rms_w
    q/k/v = xn @ Wq/Wk/Wv            (16 heads x 128 head dim)
    att = causal_softmax(q k^T / sqrt(2048)) @ v
    out[b] = att @ Wo + x[b]

Sharding (8 NeuronCores): tensor-parallel over heads x data-parallel over
batch.  Core c handles batch b = c // 4 and head-group i = c % 4 (4 heads,
512 columns of Wq/Wk/Wv, 512 rows of Wo).  Each core computes a partial
output  att_i @ Wo_i  for its batch element; the host sums the 4 partials
per batch and adds the residual.

fp8 strategy: the K-heavy matmuls (QKV projections, PV, softmax
denominator, o-proj, rms stats) run in fp8e4 DoubleRow perf mode, which
contracts two 128-row K-planes per instruction at ~1 cycle per output
column (2x the bf16 K-throughput on hw).  Operands are laid out as
[128, 2, N] tiles (partition, k-plane, free); host-side numpy pre-packs
x^T / Wq / Wk / Wv / Wo into matching [n, 128, 2, C] dram layouts.
Scores stay bf16 (no DoubleRow gain at K=128).

Scale folding keeps every fp8 operand in e4m3's normal range:
  - Wq/Wk/Wv are scaled x64 on host; rstd/64 is folded into the PSUM
    evacuation of q/k/v (q/k emerge in bf16 for the scores matmul).
  - the softmax score scale 1/sqrt(2048) rides the Exp activation.
  - the denominator ones-matmul uses 0.125-valued "ones" so attn gets
    evacuated as 8*att (fp8-friendly); Wo is scaled x8 on host and the
    o-proj PSUM (64x true) is scaled back by 1/64 during evacuation.

Scheduling: the kernel software-pipelines the ACT-bound attention inner
loop (exp ~1us per t-pair) against background PE work.  Emission order:
rms stats + buffered q0 projection chunks overlap the x DMA; per-chunk
attention runs with 1-pair scores lookahead; later-head projections,
v-projection tiles and the previous chunk's o-proj are woven between
attention heads so the Tensor engine never starves.  PSUM (8 banks) is
split into three pools: 2x2-bank score pairs, 2x1 attention accumulators
(att/den for one head), 2x1 background (proj/v/o accumulation).
Output partials are written bf16; the host sums in fp32 with residual.
"""

import numpy as np
import ml_dtypes

S = 2048          # sequence length
HID = 2048        # hidden dim
NH = 16           # total heads
DH = 128          # head dim
TP = 4            # head-group shards
DP = 2            # batch shards
KSH = HID // TP   # per-core key-dim shard (512)
NHS = KSH // DH   # heads per core (4)
NT = S // 128     # 128-row tiles along s/t (16)
NSC = S // 512    # 512-wide chunks along s (4)
NP8 = HID // 256  # k-pair planes for DoubleRow (8)
EPS = 1e-5
WS = 64.0         # host scale on Wq/Wk/Wv for fp8 range
WOS = 8.0         # host scale on Wo; attn carries the other 8x
SCORE_SCALE = 1.0 / float(np.sqrt(2048.0))

_STATE = {}


def _build_nc():
    from contextlib import ExitStack

    import concourse.bacc as bacc
    import concourse.tile as tile
    from concourse import mybir

    F32 = mybir.dt.float32
    BF = mybir.dt.bfloat16
    FP8 = mybir.dt.float8e4
    AF = mybir.ActivationFunctionType
    DR = mybir.MatmulPerfMode.DoubleRow

    nc = bacc.Bacc("TRN2")
    xt = nc.dram_tensor("xt", [NP8, 128, 2, S], FP8, kind="ExternalInput")
    wq = nc.dram_tensor("wq", [NP8, 128, 2, KSH], FP8, kind="ExternalInput")
    wk = nc.dram_tensor("wk", [NP8, 128, 2, KSH], FP8, kind="ExternalInput")
    wv = nc.dram_tensor("wv", [NP8, 128, 2, KSH], FP8, kind="ExternalInput")
    wo = nc.dram_tensor("wo", [2, 128, 2, HID], FP8, kind="ExternalInput")
    out = nc.dram_tensor("out", [S, HID], BF, kind="ExternalOutput")

    with tile.TileContext(nc) as tc, ExitStack() as ctx:
        misc = ctx.enter_context(tc.tile_pool(name="misc", bufs=1))
        qt_pool = ctx.enter_context(tc.tile_pool(name="qt", bufs=NHS))
        kt_pool = ctx.enter_context(tc.tile_pool(name="kt", bufs=NHS))
        v_pool = ctx.enter_context(tc.tile_pool(name="v", bufs=NT // 2))
        attn_pool = ctx.enter_context(tc.tile_pool(name="attn", bufs=2))
        probs_pool = ctx.enter_context(tc.tile_pool(name="probs", bufs=8))
        denb_pool = ctx.enter_context(tc.tile_pool(name="denb", bufs=4))
        xnt_pool = ctx.enter_context(tc.tile_pool(name="xnt", bufs=NP8, side="right"))
        xsq_pool = ctx.enter_context(tc.tile_pool(name="xsq", bufs=3, side="right"))
        w_pool = ctx.enter_context(
            tc.tile_pool(name="wstream", bufs=3 * NP8, side="right")
        )
        wo_pool = ctx.enter_context(tc.tile_pool(name="wo", bufs=2, side="right"))
        out_pool = ctx.enter_context(tc.tile_pool(name="outp", bufs=4, side="right"))

        ones_rms = misc.tile([128, 2, 128], FP8, tag="ones_rms", name="ones_rms")
        nc.vector.memset(ones_rms, 1.0)
        ones_den = misc.tile([128, 2, 128], FP8, tag="ones_den", name="ones_den")
        nc.vector.memset(ones_den, 1.0 / WOS)
        eps_sb = misc.tile([128, 1], F32, tag="eps_sb", name="eps_sb")
        nc.vector.memset(eps_sb, EPS * WS * WS)
        # rstd_b holds rstd/64 pre-broadcast across partitions
        rstd_b = misc.tile([128, S], F32, tag="rstd_b", name="rstd_b")
        # rstd_colT[p, st] = rstd[st*128+p]/64 (per-partition layout)
        rstd_colT = misc.tile([128, NT], F32, tag="rstd_colT", name="rstd_colT")
        ident = misc.tile([128, 128], F32, tag="ident", name="ident")
        nc.vector.memset(ident, 1.0)
        nc.gpsimd.affine_select(
            out=ident, in_=ident, compare_op=mybir.AluOpType.is_equal,
            fill=0.0, base=0, channel_multiplier=1, pattern=[[-1, 128]],
        )

        # PSUM pools (8 banks): ppp = 2x 2-bank score pairs (also rms stats),
        # pp_at = att/den accumulators, pp_bg = background proj/v/o.
        ppp = ctx.enter_context(tc.tile_pool(name="ppp", bufs=2, space="PSUM"))
        pp_at = ctx.enter_context(tc.tile_pool(name="pp_at", bufs=2, space="PSUM"))
        pp_bg = ctx.enter_context(tc.tile_pool(name="pp_bg", bufs=2, space="PSUM"))

        # ---------------- phase A: x^T load, rms stats ----------------------
        xnt = []
        wq_sb, wk_sb, wv_sb = [], [], []
        ss = [ppp.tile([128, 2, 512], F32, tag="ppp", name="ss") for _ in range(2)]
        for h in range(NP8):
            t = xnt_pool.tile([128, 2, S], FP8, tag="xnt", name="xnt")
            # fine-grained 64KB chunks so the first tiles land in ~3us
            for i in range(2):
                for c in range(4):
                    nc.sync.dma_start(
                        out=t[:, i, c * 512:(c + 1) * 512],
                        in_=xt[h, :, i, c * 512:(c + 1) * 512],
                    )
            xnt.append(t)
            wt = w_pool.tile([128, 2, KSH], FP8, tag="w", name="wq")
            for i in range(2):
                nc.sync.dma_start(out=wt[:, i, :], in_=wq[h, :, i, :])
            wq_sb.append(wt)
            sq = xsq_pool.tile([128, 2, S], FP8, tag="xsq", name="xsq")
            # split squares between ACT and DVE so rstd lands sooner
            for i in range(2):
                for half in range(2):
                    sl = slice(half * 1024, (half + 1) * 1024)
                    if h < 5:
                        nc.scalar.activation(sq[:, i, sl], t[:, i, sl], AF.Square)
                    else:
                        nc.vector.tensor_mul(sq[:, i, sl], t[:, i, sl], t[:, i, sl])
            # M=128 all-ones DoubleRow stationary: every partition gets the
            # column sum of x^2, so rstd lands pre-broadcast
            for sc in range(NSC):
                nc.tensor.matmul(
                    ss[sc // 2][:, sc % 2],
                    ones_rms,
                    sq[:, :, sc * 512:(sc + 1) * 512],
                    start=(h == 0),
                    stop=(h == NP8 - 1),
                    perf_mode=DR,
                )

        # --- projection helpers (emission-order building blocks) -----------
        pool_tag = {id(ppp): "ppp", id(pp_at): "pp_at", id(pp_bg): "pp_bg"}

        def proj_chunk(w_tiles, hd, sc, pool):
            """Accumulate one [dh, 512] q/k chunk; returns its psum tile."""
            ps = pool.tile([128, 512], F32, tag=pool_tag[id(pool)], name="pq")
            for h in range(NP8):
                nc.tensor.matmul(
                    ps,
                    w_tiles[h][:, :, hd * 128:(hd + 1) * 128],
                    xnt[h][:, :, sc * 512:(sc + 1) * 512],
                    start=(h == 0),
                    stop=(h == NP8 - 1),
                    perf_mode=DR,
                )
            return ps

        def proj_evac(dst, sc, ps):
            cs = slice(sc * 512, (sc + 1) * 512)
            # fold rstd/64 (free axis here) into the evacuation
            nc.vector.tensor_mul(dst[:, cs], ps, rstd_b[:, cs])

        def v_tile(st, pool):
            """One v s-tile (natural layout) into v_sb pair plane st%2."""
            psv = pool.tile([128, 512], F32, tag=pool_tag[id(pool)], name="psv")
            for h in range(NP8):
                nc.tensor.matmul(
                    psv,
                    xnt[h][:, :, st * 128:(st + 1) * 128],
                    wv_sb[h],
                    start=(h == 0),
                    stop=(h == NP8 - 1),
                    perf_mode=DR,
                )
            # fold rstd/64 (partition axis here) into the evacuation
            nc.vector.tensor_scalar_mul(
                v_sb[st // 2][:, st % 2, :], psv, rstd_colT[:, st:st + 1]
            )

        qts = [qt_pool.tile([128, S], BF, tag="qt", name="qt") for _ in range(NHS)]
        kts = [kt_pool.tile([128, S], BF, tag="kt", name="kt") for _ in range(NHS)]
        v_sb = [
            v_pool.tile([128, 2, KSH], FP8, tag="v", name="v")
            for _ in range(NT // 2)
        ]

        # buffered q0 chunks: matmuls pace with x arrivals, evac waits rstd
        q0_ps = [proj_chunk(wq_sb, 0, sc, (pp_bg, pp_bg, pp_at, pp_at)[sc])
                 for sc in range(NSC)]

        for h in range(NP8):
            wt = w_pool.tile([128, 2, KSH], FP8, tag="w", name="wk")
            for i in range(2):
                nc.sync.dma_start(out=wt[:, i, :], in_=wk[h, :, i, :])
            wk_sb.append(wt)

        # rstd: mtmp = 64*sqrt(ms+eps); recip -> rstd/64 (all lanes)
        for sc in range(NSC):
            cs = slice(sc * 512, (sc + 1) * 512)
            mtmp = denb_pool.tile([128, 512], F32, tag="denb", name="mtmp")
            nc.scalar.activation(
                mtmp, ss[sc // 2][:, sc % 2], AF.Sqrt,
                bias=eps_sb, scale=WS * WS / HID,
            )
            nc.vector.reciprocal_approx_fast(rstd_b[:, cs], mtmp)

        for sc in range(NSC):
            proj_evac(qts[0], sc, q0_ps[sc])

        # PE-transpose rstd_b slices to per-partition rstd columns
        for st in range(NT):
            ptr = ppp.tile([128, 2, 512], F32, tag="ppp", name="ptr")
            nc.tensor.transpose(
                ptr[:, 0, 0:128], rstd_b[:, st * 128:(st + 1) * 128], ident
            )
            nc.vector.tensor_copy(rstd_colT[:, st:st + 1], ptr[:, 0, 0:1])

        for h in range(NP8):
            wt = w_pool.tile([128, 2, KSH], FP8, tag="w", name="wv")
            for i in range(2):
                nc.sync.dma_start(out=wt[:, i, :], in_=wv[h, :, i, :])
            wv_sb.append(wt)

        def proj_head(w_tiles, dst, hd, pools=(pp_bg,) * NSC):
            for sc in range(NSC):
                ps = proj_chunk(w_tiles, hd, sc, pools[sc])
                proj_evac(dst, sc, ps)

        # pre-attention: pp_at is free, so alternate pools for 4-deep lookahead
        pre_pools = (pp_bg, pp_at, pp_bg, pp_at)
        proj_head(wk_sb, kts[0], 0, pre_pools)
        proj_head(wq_sb, qts[1], 1, pre_pools)
        proj_head(wk_sb, kts[1], 1, pre_pools)
        for st in range(4):
            v_tile(st, (pp_bg, pp_at, pp_bg, pp_at)[st % 4])

        wo_sb = []
        for hp in range(2):
            wt = wo_pool.tile([128, 2, HID], FP8, tag="wo", name="wo")
            for i in range(2):
                for c in range(4):
                    nc.sync.dma_start(
                        out=wt[:, i, c * 512:(c + 1) * 512],
                        in_=wo[hp, :, i, c * 512:(c + 1) * 512],
                    )
            wo_sb.append(wt)

        # -------- attention + o-proj with background interleave -------------
        # attn pairs hold 8*att in fp8: attn_sb[hd//2][:, hd%2, :] = 8*att^T
        attn_sb = [
            attn_pool.tile([128, 2, S], FP8, tag="attn", name="attn")
            for _ in range(2)
        ]

        def attn_head(sc, hd):
            cs = slice(sc * 512, (sc + 1) * 512)
            npair = 2 * (sc + 1)
            ps_at = pp_at.tile([128, 512], F32, tag="pp_at", name="at")
            ps_dn = pp_at.tile([128, 512], F32, tag="pp_at", name="dn")
            pending = None  # (u, c0, pt) awaiting PV/den, 1-pair lookahead

            def flush():
                u, c0, pt = pending
                nc.tensor.matmul(
                    ps_at[:, c0:],
                    v_sb[u][:, :, hd * 128:(hd + 1) * 128],
                    pt[:, :, c0:],
                    start=(u == 0),
                    stop=(u == npair - 1),
                    perf_mode=DR,
                )
                nc.tensor.matmul(
                    ps_dn[:, c0:],
                    ones_den,
                    pt[:, :, c0:],
                    start=(u == 0),
                    stop=(u == npair - 1),
                    perf_mode=DR,
                )

            for u in range(npair):
                # pair-level causal truncation: cols < c0 are fully masked
                c0 = max(0, 256 * u - 512 * sc)
                ps_s = ppp.tile([128, 2, 512], F32, tag="ppp", name="ps")
                for j in range(2):
                    nc.tensor.matmul(
                        ps_s[:, j, c0:],
                        kts[hd][:, (2 * u + j) * 128:(2 * u + j + 1) * 128],
                        qts[hd][:, sc * 512 + c0:(sc + 1) * 512],
                        start=True,
                        stop=True,
                    )
                if pending is not None:
                    flush()
                pt = probs_pool.tile([128, 2, 512], FP8, tag="probs", name="pt")
                nc.scalar.activation(
                    pt[:, :, c0:], ps_s[:, :, c0:], AF.Exp, scale=SCORE_SCALE
                )
                if u >= 2 * sc:
                    # diagonal pair: zero where s < t inside a 256-wide
                    # window starting at c0 (predicate c - 128i - p >= 0)
                    nc.gpsimd.affine_select(
                        out=pt[:, :, c0:c0 + 256],
                        in_=pt[:, :, c0:c0 + 256],
                        compare_op=mybir.AluOpType.is_ge,
                        fill=0.0,
                        base=0,
                        channel_multiplier=-1,
                        pattern=[[-128, 2], [1, 256]],
                    )
                pending = (u, c0, pt)
            flush()
            denb = denb_pool.tile([128, 512], F32, tag="denb", name="denb")
            nc.vector.reciprocal_approx_fast(denb, ps_dn)
            nc.vector.tensor_mul(attn_sb[hd // 2][:, hd % 2, cs], ps_at, denb)

        def o_tile(st, act_share=False):
            ot = out_pool.tile([128, HID], BF, tag="outp", name="outp")
            for ec in range(4):
                po = pp_bg.tile([128, 512], F32, tag="pp_bg", name="po")
                for hp in range(2):
                    nc.tensor.matmul(
                        po,
                        attn_sb[hp][:, :, st * 128:(st + 1) * 128],
                        wo_sb[hp][:, :, ec * 512:(ec + 1) * 512],
                        start=(hp == 0),
                        stop=(hp == 1),
                        perf_mode=DR,
                    )
                es = slice(ec * 512, (ec + 1) * 512)
                # psum holds 64x the true partial; scale back here.  In the
                # tail (exp stream exhausted) ACT takes half the evacuations.
                if act_share and ec % 2:
                    nc.scalar.mul(ot[:, es], po, 1.0 / WS)
                else:
                    nc.vector.tensor_scalar_mul(ot[:, es], po, 1.0 / WS)
                nc.sync.dma_start(
                    out=out[st * 128:(st + 1) * 128, es], in_=ot[:, es]
                )

        # chunk 0: background = heads 2/3 projections (each head's q/k must
        # be fully emitted before its own attention -- PE runs in-order)
        attn_head(0, 0)
        proj_head(wq_sb, qts[2], 2)
        attn_head(0, 1)
        proj_head(wk_sb, kts[2], 2)
        attn_head(0, 2)
        proj_head(wq_sb, qts[3], 3)
        proj_head(wk_sb, kts[3], 3)
        attn_head(0, 3)
        for st in range(0, 2):
            o_tile(st)
        for st in range(4, 8):
            v_tile(st, pp_bg)
        for st in range(2, 4):
            o_tile(st)
        # chunk 1: background = v tiles 8..11
        attn_head(1, 0)
        for st in range(8, 10):
            v_tile(st, pp_bg)
        attn_head(1, 1)
        for st in range(10, 12):
            v_tile(st, pp_bg)
        attn_head(1, 2)
        attn_head(1, 3)
        # chunk 2: background = v tiles 12..15 + chunk 1's o-proj
        attn_head(2, 0)
        for st in range(12, 14):
            v_tile(st, pp_bg)
        attn_head(2, 1)
        for st in range(14, 16):
            v_tile(st, pp_bg)
        attn_head(2, 2)
        o_tile(4)
        o_tile(5)
        attn_head(2, 3)
        o_tile(6)
        o_tile(7)
        # chunk 3: background = chunk 2's o-proj
        attn_head(3, 0)
        o_tile(8)
        o_tile(9)
        attn_head(3, 1)
        o_tile(10)
        o_tile(11)
        attn_head(3, 2)
        attn_head(3, 3)
        for st in range(12, 16):
            o_tile(st, act_share=True)

    return nc


def get_nc():
    if "nc" not in _STATE:
        nc = _build_nc()
        nc.finalize()
        _STATE["nc"] = nc
    return _STATE["nc"]


def _pair_rows(a):
    """[256*n, C] -> [n, 128, 2, C] with [h, p, i, c] = a[256h+128i+p, c]."""
    n = a.shape[0] // 256
    return np.ascontiguousarray(
        a.reshape(n, 2, 128, a.shape[1]).transpose(0, 2, 1, 3)
    )


def make_in_maps(x, rms_w, Wq, Wk, Wv, Wo):
    """Host-side sharding: returns one input dict per core (8 cores)."""
    fp8 = ml_dtypes.float8_e4m3
    rw = rms_w.astype(np.float32)[:, None]
    wq_f = rw * Wq.astype(np.float32) * WS
    wk_f = rw * Wk.astype(np.float32) * WS
    wv_f = rw * Wv.astype(np.float32) * WS
    wo_f = Wo.astype(np.float32) * WOS
    in_maps = []
    for c in range(DP * TP):
        b, i = divmod(c, TP)
        cols = slice(i * KSH, (i + 1) * KSH)
        in_maps.append({
            "xt": _pair_rows(np.ascontiguousarray(x[b].T)).astype(fp8),
            "wq": _pair_rows(wq_f[:, cols]).astype(fp8),
            "wk": _pair_rows(wk_f[:, cols]).astype(fp8),
            "wv": _pair_rows(wv_f[:, cols]).astype(fp8),
            "wo": _pair_rows(np.ascontiguousarray(wo_f[cols, :])).astype(fp8),
        })
    return in_maps


def kernel(x, rms_w, Wq, Wk, Wv, Wo, _trace=False, _results_out=None):
    from concourse.bass_utils import run_bass_kernel_spmd

    nc = get_nc()
    in_maps = make_in_maps(x, rms_w, Wq, Wk, Wv, Wo)
    kw = {}
    if _trace:
        kw = dict(trace=True, trace_cores=list(range(DP * TP)))
    res = run_bass_kernel_spmd(
        nc, in_maps, core_ids=list(range(DP * TP)), **kw
    )
    if _results_out is not None:
        _results_out.append(res)
    out = np.empty((DP, S, HID), np.float32)
    for b in range(DP):
        acc = x[b].astype(np.float32).copy()
        for i in range(TP):
            acc += res.results[b * TP + i]["out"].astype(np.float32)
        out[b] = acc
    return out
